# revision 17
# baseline (speedup 1.0000x reference)
"""Trainium2 Bass kernel for nn_ByteSequenceEmbedder.

Packed data-parallel across 8 NeuronCores: the valid bytes of all 16
sequences (src_len ~2048 each vs T=3072 padded) are concatenated and split
at word boundaries into 8 balanced per-core streams, so each core processes
~4100 positions (9 chunks of CW<=512) instead of 2x3072 padded positions.

Stream layout per core: segments of real bytes, with
  - 2-col embed halo around mid-sequence splits (outputs discarded),
  - 2 zero cols after each sequence end (first = the reference's position
    src_len, whose conv0/highway value feeds conv1 at src_len-1),
  - a per-column multiplicative mask (cmask) applied in place to the conv1
    input buffer, zeroing the column before each sequence start (reproduces
    conv SAME zero-padding; the zeroed columns' own outputs are discarded).

v3 dataflow per core (channels-on-partitions; bufA/B/C [128, 4, SPU] bf16,
P a single [128, 4, SPUF] fp8-e4m3 cast buffer):
  embed   : tokens host-remapped to tok-4 in [0,255] (pads -> -4, no
            match); 2 one-hot matmuls vs per-partition iota; the BPE-mark
            row is row 0 of the remapped table so a [1,w] add of the bpe
            mask onto oh1 partition 0 replaces a third matmul.
  conv0   : bf16 (fp8 here costs too much accuracy: its operands carry the
            raw signal); merged ReLU evac, cast -> P (x1c).
  highway : all 4 layers fp8 DoubleRow, weights/activations e4m3 with
            power-2 scales folded into the merged ReLU/Sigmoid evacs;
            merged sub/mul/add combine on DVE; each layer's output cast
            overwrites its own rhs region of P after the matmuls read it
            (x1c -> x2 -> x3 -> x4 rotation in one buffer).
  conv1   : bf16 (the dominant fp8 error contributor: 1536-long
            contraction feeding the residual ungated), 48 matmuls.
  pool    : ragged word max-pool as masked shifted max (additive -1e30
            bf16 masks); per-slab adds (s2 on GpSimd), merged maxes.
  proj    : bf16 matmuls over all stream cols; merged evac into dead bufC
            cols; host gathers word-start cols.

All stages are emitted as a chunk-wavefront (stage s of chunk n alongside
stage s+1 of chunk n-1, ...) so the per-engine FIFOs interleave every
stage and no engine waits out another's whole layer (layer-major emission
serialized PE behind ACT at every layer boundary).

Zero biases (true for this model) enable the merged evacs; a per-m-chunk
evac path with bias APs is kept for the general case.
"""
import numpy as np

import concourse.bacc as bacc
import concourse.tile as tile
import concourse.mybir as mybir

BSZ, NW, T = 16, 1024, 3072
BED, WED = 128, 512
N_CORES = 8
BF16 = mybir.dt.bfloat16
F32 = mybir.dt.float32
F8 = mybir.dt.float8e4
_F8_NP = mybir.dt.np(mybir.dt.float8e4)
_BF16_NP = mybir.dt.np(BF16)

# static power-2 scales for the fp8 activation casts (validated ranges with
# >=2x margin below the TRN e4m3 240 cap)
KX1 = 11   # conv0 out (max ~0.052 -> 105)
KX2 = 11   # hw0l0 out (max ~0.030 -> 60)
KX3 = 12   # conv1+res out (max ~0.018 -> 71)
KX4 = 13   # hw1l0 out (max ~0.010 -> 74)

NEG_BIG = -1e30

_CACHE = {}


def _plan_packing(pool_lengths):
    """Balanced word-aligned split of the global byte stream across cores."""
    pl = np.asarray(pool_lengths, np.int64)
    assert pl.max() <= 3, "pool lengths > 3 unsupported"
    cum = np.cumsum(pl, axis=1)
    src = cum[:, -1]
    starts = cum - pl
    total = int(src.sum())
    seq_base = np.zeros(BSZ, np.int64)
    seq_base[1:] = np.cumsum(src)[:-1]
    gstarts = (starts + seq_base[:, None]).ravel()
    wcuts = [int(np.searchsorted(gstarts, round(total * c / N_CORES)))
             for c in range(N_CORES + 1)]
    wcuts[0], wcuts[-1] = 0, BSZ * NW

    cores = []
    max_need = 0
    for c in range(N_CORES):
        w0, w1 = wcuts[c], wcuts[c + 1]
        segs = []
        need = 0
        w = w0
        while w < w1:
            b = w // NW
            we = min(w1, (b + 1) * NW)
            lw0, lw1 = w % NW, ((we - 1) % NW) + 1
            t0, t1 = int(starts[b, lw0]), int(cum[b, lw1 - 1])
            if t1 > t0:
                segs.append((b, lw0, lw1, t0, t1))
                if t0 > 0:
                    need += min(2, t0)
                need += t1 - t0
                need += 2 if t1 == int(src[b]) else min(2, int(src[b]) - t1)
            w = we
        cores.append((segs, (w0, w1)))
        max_need = max(max_need, need)

    S8 = -(-max_need // 8) * 8
    NCH = max(1, -(-S8 // 512))
    CWmax = min(512, -(-S8 // (NCH * 8)) * 8)
    while CWmax * NCH < S8:
        NCH += 1
        CWmax = min(512, -(-S8 // (NCH * 8)) * 8)
    widths = [CWmax] * NCH
    return cores, tuple(widths), CWmax * NCH, (pl, cum, starts, src)


def _build_program(widths, kwhs, zb):
    NCH = len(widths)
    CW = max(widths)
    offs = [0]
    for w in widths:
        offs.append(offs[-1] + w)
    S = offs[-1]
    SPU = S + 16         # bf16 pitch: cols 0-7 zero, data 8..8+S, 8 zero cols
    SPUF = S + 24        # fp8 pitch; %16==0 for DoubleRow APs
    assert SPUF % 16 == 0
    nc = bacc.Bacc("TRN2", target_bir_lowering=False, debug=False)

    def dram_in(name, shape, dt):
        return nc.dram_tensor(name, shape, dt, kind="ExternalInput").ap()

    emb_lhs = dram_in("emb_lhs", [128, 2 * 128], BF16)   # remapped tok_emb
    iota_c = dram_in("iota_c", [128, 2], F32)            # per-partition iota
    w_c0 = dram_in("w_c0", [128, 3 * WED], BF16)         # [ci, k*512+co]
    w_c1 = dram_in("w_c1", [128, 48 * 128], BF16)        # [(m*3+k)*4+q]
    w_hw = dram_in("w_hw", [128, 128, 128], F8)          # 4 hw layers DR pairs
    w_pr = dram_in("w_pr", [128, 4 * WED], BF16)         # [q*512+co]
    tok_bc = dram_in("tok_bc", [128, S], BF16)           # remapped tokens bcast
    bpe_row = dram_in("bpe_row", [1, S], BF16)           # bpe mask 0/1
    a_msk = dram_in("a_msk", [128, 2 * S], BF16)         # pooling additive masks
    c_msk = dram_in("c_msk", [128, S], BF16)             # conv1 col mask 0/1
    if not zb:
        b_c0 = dram_in("b_c0", [128, 4], F32)
        b_c1 = dram_in("b_c1", [128, 4], F32)
        b_hw = dram_in("b_hw", [128, 4 * 8], F32)        # [bl*8 + m]
        b_pr = dram_in("b_pr", [128, 4], F32)

    SOUT = S + ((80 - S) % 512)
    out = nc.dram_tensor("out", [WED, SOUT], BF16, kind="ExternalOutput").ap()

    RELU = mybir.ActivationFunctionType.Relu
    SIGM = mybir.ActivationFunctionType.Sigmoid
    COPY = mybir.ActivationFunctionType.Copy
    MAX = mybir.AluOpType.max
    ADD = mybir.AluOpType.add
    SUB = mybir.AluOpType.subtract
    MUL = mybir.AluOpType.mult
    ISEQ = mybir.AluOpType.is_equal
    DR = mybir.MatmulPerfMode.DoubleRow

    # per-hw-layer evac descale: psum holds 2^(kwh+KXin) * true value
    KXIN = (KX1, KX2, KX3, KX4)
    hw_scale = [float(2.0 ** -(kwhs[l] + KXIN[l])) for l in range(4)]

    with tile.TileContext(nc) as tc:
        with tc.tile_pool(name="wp", bufs=1) as wp, \
             tc.tile_pool(name="ap", bufs=1) as apool, \
             tc.tile_pool(name="tp", bufs=3) as tp, \
             tc.tile_pool(name="pp", bufs=8, space="PSUM") as pp:

            # ---- HAM warm-up: PE activity from t~0 ----
            wu = wp.tile([128, 512], BF16)
            nc.vector.memset(wu[:], 0)
            for _ in range(100):
                wps = pp.tile([128, 1, 512], F32, tag="ps", name="wps")
                nc.tensor.matmul(out=wps[:, 0, 0:CW], lhsT=wu[:, 0:128],
                                 rhs=wu[:, 0:CW], start=True, stop=True)

            # ---- activation buffers ----
            def act_buf(tag, pitch, dt):
                b = apool.tile([128, 4, pitch], dt, tag=tag, name=tag)
                for q in range(4):
                    nc.vector.memset(b[:, q, 0:8], 0)
                    nc.vector.memset(b[:, q, 8 + S:pitch], 0)
                return b

            bufA = act_buf("actA", SPU, BF16)
            bufB = act_buf("actB", SPU, BF16)
            bufC = act_buf("actC", SPU, BF16)
            P = act_buf("pf8", SPUF, F8)
            # aliases: tok lives in bufC slab 0 (dead before hw0l1 writes
            # bufC); x0 lives in bufB slab 0 (dead before hw0l0 writes bufB)
            t_tok = bufC[:, 0, 8:8 + S]
            x0 = bufB[:, 0, :]

            # first token chunk ahead of the weight loads (critical path)
            nc.sync.dma_start(out=bufC[:, 0, 8:8 + widths[0]],
                              in_=tok_bc[:, 0:widths[0]])

            # ---- load weights/biases once ----
            t_emb = wp.tile([128, 2 * 128], BF16)
            t_iota = wp.tile([128, 2], F32)
            t_wc0 = wp.tile([128, 3 * WED], BF16)
            t_wc1 = wp.tile([128, 48 * 128], BF16)
            t_whw = wp.tile([128, 128, 128], F8)
            t_wpr = wp.tile([128, 4 * WED], BF16)
            t_bpe = apool.tile([1, S], BF16, tag="bpe", name="t_bpe")
            t_am = apool.tile([128, 2 * S], BF16, tag="am", name="t_am")
            t_cm = apool.tile([128, S], BF16, tag="cm", name="t_cm")
            loads = [(t_iota, iota_c), (t_emb, emb_lhs), (t_wc0, w_c0),
                     (t_whw, w_hw), (t_wc1, w_c1), (t_wpr, w_pr)]
            if not zb:
                t_bc0 = wp.tile([128, 4], F32)
                t_bc1 = wp.tile([128, 4], F32)
                t_bhw = wp.tile([128, 4 * 8], F32)
                t_bpr = wp.tile([128, 4], F32)
                loads += [(t_bc0, b_c0), (t_bc1, b_c1), (t_bhw, b_hw),
                          (t_bpr, b_pr)]
            for t, d in loads:
                nc.sync.dma_start(out=t[:], in_=d[:])
            # big weight tensors on their own queues, ordered by first need
            nc.sync.dma_start(out=t_whw[:, 0:64, :], in_=w_hw[:, 0:64, :])
            nc.gpsimd.dma_start(out=t_whw[:, 64:128, :], in_=w_hw[:, 64:128, :])
            nc.gpsimd.dma_start(out=t_wc1[:], in_=w_c1[:])
            nc.sync.dma_start(out=t_wpr[:], in_=w_pr[:])
            for n in range(1, NCH):
                nc.scalar.dma_start(out=bufC[:, 0, 8 + offs[n]:8 + offs[n + 1]],
                                    in_=tok_bc[:, offs[n]:offs[n + 1]])
            nc.scalar.dma_start(out=t_bpe[:], in_=bpe_row[:])
            nc.scalar.dma_start(out=t_cm[:], in_=c_msk[:])
            nc.scalar.dma_start(out=t_am[:], in_=a_msk[:])

            def evac1(t, dst2, func, scale, btile, bcol):
                """one 1-bank PSUM tile -> SBUF [128,1,w]."""
                if zb:
                    nc.scalar.activation(out=dst2, in_=t[:, 0, 0:dst2.shape[-1]],
                                         func=func, bias=0.0, scale=scale)
                else:
                    nc.scalar.activation(
                        out=dst2, in_=t[:, 0, 0:dst2.shape[-1]], func=func,
                        bias=btile[:, bcol:bcol + 1], scale=scale)

            # ---------------- per-chunk stage emitters ----------------
            def embed(n):
                lo, hi = offs[n], offs[n + 1]
                w = widths[n]
                oh1 = tp.tile([128, CW], BF16, tag="oh", name="oh1", bufs=2)
                oh2 = tp.tile([128, CW], BF16, tag="oh", name="oh2", bufs=2)
                tb = t_tok[:, lo:hi]
                nc.vector.tensor_scalar(out=oh1[:, 0:w], in0=tb,
                                        scalar1=t_iota[:, 0:1],
                                        scalar2=None, op0=ISEQ)
                nc.vector.tensor_scalar(out=oh2[:, 0:w], in0=tb,
                                        scalar1=t_iota[:, 1:2],
                                        scalar2=None, op0=ISEQ)
                # BPE mark: remapped row 0 holds tok_emb[4]
                nc.vector.tensor_tensor(out=oh1[0:1, 0:w], in0=oh1[0:1, 0:w],
                                        in1=t_bpe[:, lo:hi], op=ADD)
                ps = pp.tile([128, 1, 512], F32, tag="ps", name="ps")
                nc.tensor.matmul(out=ps[:, 0, 0:w], lhsT=t_emb[:, 0:128],
                                 rhs=oh1[:, 0:w], start=True, stop=False)
                nc.tensor.matmul(out=ps[:, 0, 0:w], lhsT=t_emb[:, 128:256],
                                 rhs=oh2[:, 0:w], start=False, stop=True)
                nc.scalar.activation(out=x0[:, 8 + lo:8 + hi],
                                     in_=ps[:, 0, 0:w], func=COPY,
                                     bias=0.0, scale=1.0)

            def conv0(n):
                lo, w = offs[n], widths[n]
                dst = bufA[:, :, 8 + lo:8 + lo + w]
                for m in range(4):
                    ps = pp.tile([128, 1, 512], F32, tag="ps", name="ps")
                    for k in range(3):
                        nc.tensor.matmul(
                            out=ps[:, 0, 0:w],
                            lhsT=t_wc0[:, k * WED + m * 128:k * WED + (m + 1) * 128],
                            rhs=x0[:, lo + 7 + k:lo + 7 + k + w],
                            start=(k == 0), stop=(k == 2))
                    evac1(ps, dst[:, m:m + 1, :], RELU, 1.0,
                          None if zb else t_bc0, m)
                # x1c fp8 cast for hw0l0's DR rhs
                dve_cast(P[:, :, 8 + lo:8 + lo + w], dst, KX1)

            def act_cast(o, i, kx):
                nc.scalar.activation(out=o, in_=i, func=COPY, bias=0.0,
                                     scale=float(2.0 ** kx))

            def dve_cast(o, i, kx):
                nc.vector.tensor_scalar(out=o, in0=i,
                                        scalar1=float(2.0 ** kx),
                                        scalar2=None, op0=MUL)

            def highway_layer(n, X, Y, bl, cast_kx=None, cast_eng=None):
                """Y = g*relu(h) + (1-g)*X, fp8-DR matmuls reading P;
                optionally cast Y to fp8, overwriting this chunk's cols of
                P (its own x-input region, dead after the matmuls above)."""
                lo, w = offs[n], widths[n]
                hi = lo + w
                h_t = tp.tile([128, 4, CW], BF16, tag="hg", name="h_t", bufs=3)
                g_t = tp.tile([128, 4, CW], BF16, tag="hg", name="g_t", bufs=3)
                xs = X[:, :, 8 + lo:8 + hi]
                for m in range(8):
                    dst_t = h_t if m < 4 else g_t
                    func = RELU if m < 4 else SIGM
                    boff = bl * 8 + m
                    ps = pp.tile([128, 1, 512], F32, tag="ps", name="ps")
                    for qp in range(2):
                        j = bl * 32 + (m * 2 + qp) * 2
                        nc.tensor.matmul(
                            out=ps[:, 0, 0:w],
                            lhsT=t_whw[:, j:j + 2, :],
                            rhs=P[:, 2 * qp:2 * qp + 2, 8 + lo:8 + hi],
                            start=(qp == 0), stop=(qp == 1),
                            perf_mode=DR)
                    d2 = dst_t[:, (m % 4):(m % 4) + 1, 0:w]
                    if zb and m < 4 and bl in (0, 2):
                        # relu evac on DVE: (psum max 0) * scale
                        nc.vector.tensor_scalar(out=d2, in0=ps[:, 0:1, 0:w],
                                                scalar1=0.0,
                                                scalar2=hw_scale[bl],
                                                op0=MAX, op1=MUL)
                    else:
                        evac1(ps, d2, func, hw_scale[bl],
                              None if zb else t_bhw, boff)
                nc.vector.tensor_tensor(out=h_t[:, :, 0:w], in0=h_t[:, :, 0:w],
                                        in1=xs, op=SUB)
                nc.vector.tensor_tensor(out=h_t[:, :, 0:w], in0=h_t[:, :, 0:w],
                                        in1=g_t[:, :, 0:w], op=MUL)
                ys = Y[:, :, 8 + lo:8 + hi]
                nc.vector.tensor_tensor(out=ys, in0=h_t[:, :, 0:w], in1=xs,
                                        op=ADD)
                if cast_kx is not None:
                    cast_eng(P[:, :, 8 + lo:8 + hi], ys, cast_kx)

            def cmask(n):
                # zero the conv-SAME boundary cols of bufC in place (the
                # masked cols' own outputs are discarded downstream)
                lo, hi = offs[n], offs[n] + widths[n]
                for c in range(4):
                    nc.vector.tensor_tensor(out=bufC[:, c, 8 + lo:8 + hi],
                                            in0=bufC[:, c, 8 + lo:8 + hi],
                                            in1=t_cm[:, lo:hi], op=MUL)

            def conv1(n):
                lo, w = offs[n], widths[n]
                r_t = tp.tile([128, 4, CW], BF16, tag="hg", name="r_t", bufs=3)
                for m in range(4):
                    ps = pp.tile([128, 1, 512], F32, tag="ps", name="ps")
                    i = 0
                    for k in range(3):
                        for q in range(4):
                            nc.tensor.matmul(
                                out=ps[:, 0, 0:w],
                                lhsT=t_wc1[:, ((m * 3 + k) * 4 + q) * 128:
                                           ((m * 3 + k) * 4 + q) * 128 + 128],
                                rhs=bufC[:, q, 7 + lo + k:7 + lo + k + w],
                                start=(i == 0), stop=(i == 11))
                            i += 1
                    evac1(ps, r_t[:, m:m + 1, 0:w], RELU, 1.0,
                          None if zb else t_bc1, m)
                dst = bufA[:, :, 8 + lo:8 + lo + w]
                nc.vector.tensor_tensor(out=dst, in0=r_t[:, :, 0:w],
                                        in1=bufC[:, :, 8 + lo:8 + lo + w],
                                        op=ADD)
                act_cast(P[:, :, 8 + lo:8 + lo + w], dst, KX3)

            def pool_chunk(n):
                """msel[t] = max(Y[t], Y[t+1]+A1[t], Y[t+2]+A2[t]) for chunk
                n (Y = bufC, msel = bufA). Reads one/two cols into chunk
                n+1, so emitted only after hw1l1's combine of chunk n+1."""
                lo, w = offs[n], widths[n]
                hi = lo + w
                s1 = tp.tile([128, 4, CW], BF16, tag="s", name="s1", bufs=3)
                s2 = tp.tile([128, 4, CW], BF16, tag="s", name="s2", bufs=3)
                eng2 = nc.vector if n >= NCH - 2 else nc.gpsimd
                for c in range(4):
                    # +1 col read is 2B-misaligned -> DVE 1x; +2 is aligned
                    nc.vector.tensor_tensor(out=s1[:, c, 0:w],
                                            in0=bufC[:, c, 9 + lo:9 + hi],
                                            in1=t_am[:, lo:hi], op=ADD)
                    eng2.tensor_tensor(out=s2[:, c, 0:w],
                                       in0=bufC[:, c, 10 + lo:10 + hi],
                                       in1=t_am[:, S + lo:S + hi], op=ADD)
                nc.vector.tensor_tensor(out=s1[:, :, 0:w], in0=s1[:, :, 0:w],
                                        in1=s2[:, :, 0:w], op=MAX)
                nc.vector.tensor_tensor(out=bufA[:, :, 8 + lo:8 + hi],
                                        in0=s1[:, :, 0:w],
                                        in1=bufC[:, :, 8 + lo:8 + hi], op=MAX)

            def proj_chunk(n):
                lo, w = offs[n], widths[n]
                hi = lo + w
                # evac into dead bufC cols (chunk n cols are pool-read-done)
                dst = bufC[:, :, 8 + lo:8 + hi]
                for m in range(4):
                    ps = pp.tile([128, 1, 512], F32, tag="ps", name="ps")
                    for q in range(4):
                        nc.tensor.matmul(
                            out=ps[:, 0, 0:w],
                            lhsT=t_wpr[:, q * WED + m * 128:q * WED + (m + 1) * 128],
                            rhs=bufA[:, q, 8 + lo:8 + hi],
                            start=(q == 0), stop=(q == 3))
                    if zb:
                        nc.scalar.activation(out=dst[:, m:m + 1, :],
                                             in_=ps[:, 0:1, 0:w], func=COPY,
                                             bias=0.0, scale=1.0)
                    else:
                        nc.vector.tensor_scalar(out=dst[:, m, :],
                                                in0=ps[:, 0, 0:w],
                                                scalar1=t_bpr[:, m:m + 1],
                                                scalar2=None, op0=ADD)
                for m in range(4):
                    dq = nc.sync if (n * 4 + m) % 2 == 0 else nc.scalar
                    dq.dma_start(out=out[m * 128:(m + 1) * 128, lo:hi],
                                 in_=bufC[:, m, 8 + lo:8 + hi])

            # ---------------- chunk-wavefront emission ----------------
            STAGES = [
                (0, embed),
                (2, conv0),
                (3, lambda n: highway_layer(n, bufA, bufB, 0,
                                            cast_kx=KX2, cast_eng=dve_cast)),
                (4, lambda n: (highway_layer(n, bufB, bufC, 1), cmask(n))),
                (6, conv1),
                (7, lambda n: highway_layer(n, bufA, bufB, 2,
                                            cast_kx=KX4, cast_eng=dve_cast)),
                (8, lambda n: highway_layer(n, bufB, bufC, 3)),
                (9, pool_chunk),
                (10, proj_chunk),
            ]
            for step in range(NCH + 10):
                with nc.named_scope(f"s{step:02d}"):
                    for lag, fn in STAGES:
                        n = step - lag
                        if 0 <= n < NCH:
                            fn(n)

    nc.compile()
    return nc


def _prep_inputs(inputs):
    """Host-side: pack + shard + convert to the kernel's DRAM layouts."""
    byte_tokens = np.asarray(inputs["byte_tokens"], np.int64)
    bpe_mask = np.asarray(inputs["bpe_mask"], bool)
    pool_lengths = np.asarray(inputs["pool_lengths"], np.int64)
    tok_emb = np.asarray(inputs["tok_emb"], np.float32)

    cores, widths, S, (pl, cum, starts, src) = _plan_packing(pool_lengths)

    def bf(x):
        return np.ascontiguousarray(np.asarray(x, np.float32).astype(_BF16_NP))

    conv0_W = np.asarray(inputs["conv0_W"], np.float32)   # [3,128,512]
    conv1_W = np.asarray(inputs["conv1_W"], np.float32)   # [3,512,512]
    hw0_W = np.asarray(inputs["hw0_W"], np.float32)       # [2,1024,512]
    hw1_W = np.asarray(inputs["hw1_W"], np.float32)
    proj_W = np.asarray(inputs["proj_W"], np.float32)     # [512,512]

    w_c0 = bf(conv0_W.transpose(1, 0, 2).reshape(128, 3 * WED))
    w_c1 = np.empty((128, 48, 128), np.float32)
    for m in range(4):
        for k in range(3):
            for q in range(4):
                w_c1[:, (m * 3 + k) * 4 + q, :] = \
                    conv1_W[k, q * 128:(q + 1) * 128, m * 128:(m + 1) * 128]
    w_c1 = bf(w_c1.reshape(128, 48 * 128))

    hw_blocks = [hw0_W[0], hw0_W[1], hw1_W[0], hw1_W[1]]   # [1024, 512] each
    kwhs = []
    w_hw = np.empty((128, 4, 32, 128), np.float32)
    for bl, W in enumerate(hw_blocks):
        kwh = int(np.floor(np.log2(128.0 / max(np.abs(W).max(), 1e-30))))
        kwhs.append(kwh)
        for m in range(8):
            for q in range(4):
                w_hw[:, bl, m * 4 + q, :] = W[m * 128:(m + 1) * 128,
                                              q * 128:(q + 1) * 128].T * 2.0 ** kwh
    w_hw = np.ascontiguousarray(w_hw.reshape(128, 128, 128).astype(_F8_NP))
    w_pr = bf(proj_W.T.reshape(4, 128, WED).transpose(1, 0, 2).reshape(128, 4 * WED))

    def colchunks(b):  # [512] -> [128, 4]
        return np.ascontiguousarray(np.asarray(b, np.float32).reshape(4, 128).T)

    zb = all(not np.any(np.asarray(inputs[k], np.float32))
             for k in ("conv0_b", "conv1_b", "hw0_b", "hw1_b", "proj_b"))

    # remapped embedding: row r holds tok_emb[r+4] (r in 0..255); row 0 is
    # tok_emb[4] = the BPE mark row
    emb_lhs = np.zeros((128, 2 * 128), np.float32)
    emb_lhs[:, 0:128] = tok_emb[4:132]
    emb_lhs[:, 128:256] = tok_emb[132:260]
    emb_lhs = bf(emb_lhs)
    iota_c = np.empty((128, 2), np.float32)
    p = np.arange(128)
    iota_c[:, 0] = p
    iota_c[:, 1] = 128 + p

    shared = dict(emb_lhs=emb_lhs, iota_c=iota_c,
                  w_c0=w_c0, w_c1=w_c1, w_hw=w_hw, w_pr=w_pr)
    if not zb:
        bhw = np.empty((128, 4, 8), np.float32)
        for bl, (blk, lay) in enumerate((("hw0_b", 0), ("hw0_b", 1),
                                         ("hw1_b", 0), ("hw1_b", 1))):
            b = np.asarray(inputs[blk], np.float32)[lay]      # [1024]
            bhw[:, bl, 0:4] = b[:512].reshape(4, 128).T
            bhw[:, bl, 4:8] = b[512:1024].reshape(4, 128).T
        shared.update(b_c0=colchunks(inputs["conv0_b"]),
                      b_c1=colchunks(inputs["conv1_b"]),
                      b_hw=np.ascontiguousarray(bhw.reshape(128, 32)),
                      b_pr=colchunks(inputs["proj_b"]))

    in_maps = []
    meta = []
    for core in range(N_CORES):
        segs, _wr = cores[core]
        tok = np.full(S, -4.0, np.float32)    # remapped: tok-4; pads -> -4
        bpe = np.zeros(S, np.float32)
        a1 = np.full(S, NEG_BIG, np.float32)
        a2 = np.full(S, NEG_BIG, np.float32)
        cmk = np.zeros(S, np.float32)
        wrows, wcols = [], []
        pos = 0

        def enc(v):
            v = np.asarray(v, np.int64)
            return np.where(v == 0, -4, v - 4).astype(np.float32)

        for (b, lw0, lw1, t0, t1) in segs:
            if t0 > 0:
                hl = min(2, t0)
                tok[pos:pos + hl] = enc(byte_tokens[b, t0 - hl:t0])
                bpe[pos:pos + hl] = bpe_mask[b, t0 - hl:t0]
                cmk[pos:pos + hl] = 1.0
                pos += hl
            body = pos
            nb = t1 - t0
            tok[pos:pos + nb] = enc(byte_tokens[b, t0:t1])
            bpe[pos:pos + nb] = bpe_mask[b, t0:t1]
            cmk[pos:pos + nb] = 1.0
            lw = np.arange(lw0, lw1)
            ln = pl[b, lw0:lw1]
            wst = starts[b, lw0:lw1] - t0 + body
            nz = ln > 0
            a1[wst[nz]] = np.where(ln[nz] > 1, 0.0, NEG_BIG)
            a2[wst[nz]] = np.where(ln[nz] > 2, 0.0, NEG_BIG)
            wrows.append(b * NW + lw[nz])
            wcols.append(wst[nz])
            pos += nb
            if t1 == int(src[b]):
                cmk[pos] = 1.0      # gap1: reference position src_len
                pos += 2
            else:
                hr = min(2, int(src[b]) - t1)
                tok[pos:pos + hr] = enc(byte_tokens[b, t1:t1 + hr])
                bpe[pos:pos + hr] = bpe_mask[b, t1:t1 + hr]
                cmk[pos] = 1.0
                pos += hr
        assert pos <= S, (pos, S)

        m = dict(shared)
        m["tok_bc"] = np.ascontiguousarray(
            np.broadcast_to(tok.astype(_BF16_NP), (128, S)))
        m["bpe_row"] = bpe.astype(_BF16_NP).reshape(1, S)
        am = np.concatenate([a1, a2]).astype(_BF16_NP)
        m["a_msk"] = np.ascontiguousarray(np.broadcast_to(am, (128, 2 * S)))
        m["c_msk"] = np.ascontiguousarray(
            np.broadcast_to(cmk.astype(_BF16_NP), (128, S)))
        in_maps.append(m)
        meta.append((np.concatenate(wrows) if wrows else np.empty(0, np.int64),
                     np.concatenate(wcols) if wcols else np.empty(0, np.int64)))
    return in_maps, (meta, widths, tuple(kwhs), zb)


def kernel(**inputs) -> np.ndarray:
    from concourse.bass_utils import run_bass_kernel_spmd

    in_maps, (meta, widths, kwhs, zb) = _prep_inputs(inputs)
    key = (widths, kwhs, zb)
    if _CACHE.get("key") != key:
        _CACHE["nc"] = _build_program(widths, kwhs, zb)
        _CACHE["key"] = key
    nc = _CACHE["nc"]

    res = run_bass_kernel_spmd(nc, in_maps, list(range(N_CORES)))

    proj_b = np.asarray(inputs["proj_b"], np.float32)
    full = np.empty((BSZ * NW, WED), np.float32)
    full[:] = proj_b
    for core in range(N_CORES):
        o = np.asarray(res.results[core]["out"], np.float32)  # [512, S]
        rows, cols = meta[core]
        if len(rows):
            full[rows] = o[:, cols].T
    return full.reshape(BSZ, NW, WED)
